# revision 1
# baseline (speedup 1.0000x reference)
"""Contrastive-learning NCE loss on 8 trn2 NeuronCores (Bass/Tile).

Problem (hardcoded shapes): B=8, L=1024, D_in=512, D_feat=256, N=B*L=8192.
  emb_k = relu(feature_k @ W + b)                     [B, L, Df]
  positive = <e1,e2> + banded_diag_mean terms         [N]
  negative = logsumexp(e1 @ e2.T, axis=-1) - log(N)   [N]
  loss = mean(-positive + negative)

Sharding: token dim N split across 8 cores = one batch row each (L == N/8).
Each core computes its [1024, 8192] slab of the similarity matrix against the
full emb_2 (recomputed locally from full feature2). The host rotates feature2
per core so the core's own batch always sits at columns 0:1023 -> the SPMD
program is core-index free.

Device layout is d-major ("transposed"): embT[d, token] so the PE contracts
over d for both the projection (K=D_in) and the sim matmul (K=Df).
Matmul operands are bf16 (full PE rate); PSUM accumulation is fp32.

logsumexp per row with shift C = diag = <e1_m, e2_m> (exact for any C; C is
a member of the row so sum >= 1, and overflow would need an off-diagonal
dot 88 above the diagonal one -- impossible at these scales). ACT computes
exp(psum - C) with fused per-row accumulation; host finishes lse = C + log(S).
"""

import numpy as np
import ml_dtypes
from contextlib import ExitStack

import concourse.bass as bass
import concourse.tile as tile
from concourse import bacc, mybir
from concourse import bass_utils

dt = mybir.dt
AF = mybir.ActivationFunctionType
ALU = mybir.AluOpType

N_CORES = 8
B, L, DIN, DF = 8, 1024, 512, 256
N = B * L
KO = DIN // 128     # 4 k-tiles of the projection contraction
NDT = DF // 128     # 2 d-tiles of the embedding dim
PAD = 4             # box-filter padding (max supported positive_range)
CW = 2048           # sim-phase column group (one PSUM tile / one ACT)

_module_cache = {}

# NOTE: walrus's LDWEIGHTS-elision pass (--enable-ldw-opt) was tried to elide
# the redundant per-matmul weight loads (~100ns each on the PE front-end), but
# that pass crashes codegen (visitInstLdweights) in this toolchain build, so
# the per-matmul LDWEIGHTS cost stays.


def _box_terms(w: int):
    """Decompose window width w (odd, <= 2*PAD+1) into power-of-2 segments:
    returns [(pow, offset), ...] s.t. window = concat of segments."""
    terms, off = [], 0
    for p in (8, 4, 2, 1):
        if w >= p:
            terms.append((p, off))
            off += p
            w -= p
    assert w == 0
    return terms


def _build(r_self: int, r_tgt: int):
    nc = bacc.Bacc("TRN2", target_bir_lowering=False, debug=False, num_devices=N_CORES)

    f1t = nc.dram_tensor("f1t", [DIN, L], dt.bfloat16, kind="ExternalInput").ap()
    f2t = nc.dram_tensor("f2t", [DIN, N], dt.bfloat16, kind="ExternalInput").ap()
    w_in = nc.dram_tensor("w_in", [DIN, DF], dt.bfloat16, kind="ExternalInput").ap()
    b_in = nc.dram_tensor("b_in", [DF], dt.float32, kind="ExternalInput").ap()

    pos_main = nc.dram_tensor("pos_main", [L], dt.float32, kind="ExternalOutput").ap()
    pos_self = nc.dram_tensor("pos_self", [L], dt.float32, kind="ExternalOutput").ap()
    pos_tgt = nc.dram_tensor("pos_tgt", [L], dt.float32, kind="ExternalOutput").ap()
    s_out = nc.dram_tensor("s_out", [128, 8 * (N // CW)], dt.float32, kind="ExternalOutput").ap()

    with tile.TileContext(nc) as tc, ExitStack() as ctx:
        const = ctx.enter_context(tc.tile_pool(name="const", bufs=1))
        stage = ctx.enter_context(tc.tile_pool(name="stage", bufs=2))
        emb = ctx.enter_context(tc.tile_pool(name="emb", bufs=1))
        band = ctx.enter_context(tc.tile_pool(name="band", bufs=1))
        prodp = ctx.enter_context(tc.tile_pool(name="prodp", bufs=2))
        rows = ctx.enter_context(tc.tile_pool(name="rows", bufs=1))
        esc = ctx.enter_context(tc.tile_pool(name="esc2", bufs=2))
        small = ctx.enter_context(tc.tile_pool(name="small", bufs=2))
        mmp = ctx.enter_context(tc.tile_pool(name="mmp", bufs=2, space="PSUM"))

        # ---- constants -------------------------------------------------
        wt = const.tile([128, KO * DF], dt.bfloat16)       # W as [k%128, (ko d)]
        nc.sync.dma_start(out=wt[:].rearrange("p (ko d) -> p ko d", ko=KO),
                          in_=w_in[:].rearrange("(ko p) d -> p ko d", p=128))
        b_col = const.tile([128, NDT], dt.float32)         # bias per (d%128, dtile)
        nc.sync.dma_start(out=b_col[:], in_=b_in[:].rearrange("(d p) -> p d", p=128))
        ones_f = const.tile([128, 1], dt.float32)
        nc.vector.memset(ones_f[:], 1.0)
        ones = const.tile([128, 1], dt.bfloat16)
        nc.vector.tensor_copy(ones[:], ones_f[:])

        # ---- projection: embT[d, tok] = relu(W.T @ fT + b) -------------
        e1 = [emb.tile([128, L], dt.bfloat16, name=f"e1_{d}", tag=f"e1_{d}")
              for d in range(NDT)]
        e2 = [emb.tile([128, N], dt.bfloat16, name=f"e2_{d}", tag=f"e2_{d}")
              for d in range(NDT)]

        def project(src_ap, col0, ncols, dst, dst_col0):
            """relu-project fT columns [col0, col0+ncols) into dst[dtile][:, dst_col0...]."""
            fst = stage.tile([128, KO * ncols], dt.bfloat16, tag="fstage")
            nc.sync.dma_start(
                out=fst[:].rearrange("p (ko n) -> p ko n", ko=KO),
                in_=src_ap[:, col0:col0 + ncols].rearrange("(ko p) n -> p ko n", p=128))
            for d in range(NDT):
                ps = mmp.tile([128, ncols], dt.float32, tag="mm")
                for ko in range(KO):                 # weight-stationary inner order
                    for half in range(ncols // 512):
                        nc.tensor.matmul(
                            ps[:, half * 512:(half + 1) * 512],
                            wt[:, ko * DF + d * 128: ko * DF + d * 128 + 128],
                            fst[:, ko * ncols + half * 512: ko * ncols + (half + 1) * 512],
                            start=(ko == 0), stop=(ko == KO - 1))
                nc.vector.tensor_scalar(
                    dst[d][:, dst_col0:dst_col0 + ncols], ps[:],
                    b_col[:, d:d + 1], 0.0, ALU.add, ALU.max)

        project(f1t, 0, L, e1, 0)
        project(f2t, 0, L, e2, 0)          # own batch first (banded phase needs it)

        # ---- banded positive terms (own batch = e2 cols 0:L) -----------
        def boxsum(src_view, r, tag):
            """Return [128, L] view/tile: out[:, j] = sum_{|d|<=r} src[:, j+d] (clipped)."""
            wdt = 2 * r + 1
            pb = band.tile([128, L + 2 * PAD], dt.bfloat16, name=f"pb_{tag}",
                           tag="pb", bufs=2)
            nc.vector.memzero(pb[:])
            nc.vector.tensor_copy(pb[:, PAD:PAD + L], src_view)
            s = {1: pb}
            for p in (2, 4, 8):
                if wdt >= p:
                    sp = band.tile([128, L + 2 * PAD], dt.bfloat16, name=f"s{p}_{tag}",
                                   tag=f"s{p}")
                    h = p // 2
                    n_valid = L + 2 * PAD - p + 1
                    nc.vector.tensor_tensor(
                        sp[:, :n_valid], s[h][:, :n_valid], s[h][:, h:h + n_valid], ALU.add)
                    s[p] = sp
            terms = _box_terms(wdt)
            t0 = PAD - r
            if len(terms) == 1:
                p0, o0 = terms[0]
                return s[p0][:, t0 + o0: t0 + o0 + L]
            acc = band.tile([128, L], dt.bfloat16, name=f"box_{tag}", tag="box", bufs=6)
            p0, o0 = terms[0]
            p1, o1 = terms[1]
            nc.vector.tensor_tensor(acc[:], s[p0][:, t0 + o0: t0 + o0 + L],
                                    s[p1][:, t0 + o1: t0 + o1 + L], ALU.add)
            for p, o in terms[2:]:
                nc.vector.tensor_tensor(acc[:], acc[:], s[p][:, t0 + o: t0 + o + L], ALU.add)
            return acc[:]

        def reduce_group(pairs, out_dram, tag):
            """out_dram[j] = sum over pairs (a,b) and d of (a*b)[d, j]."""
            row = rows.tile([1, L], dt.float32, tag=f"row_{tag}")
            for half in range(L // 512):
                rp = mmp.tile([1, 512], dt.float32, tag="mm", name=f"rp_{tag}_{half}")
                for gi, (a_view, b_view) in enumerate(pairs):
                    prod = prodp.tile([128, 512], dt.bfloat16, tag="prod")
                    nc.vector.tensor_tensor(
                        prod[:], a_view[:, half * 512:(half + 1) * 512],
                        b_view[:, half * 512:(half + 1) * 512], ALU.mult)
                    nc.tensor.matmul(rp[:], ones[:], prod[:],
                                     start=(gi == 0), stop=(gi == len(pairs) - 1))
                nc.vector.tensor_copy(row[:, half * 512:(half + 1) * 512], rp[:])
            nc.sync.dma_start(out=out_dram[:].rearrange("(one n) -> one n", one=1), in_=row[:])

        e2L = [e2[d][:, 0:L] for d in range(NDT)]
        reduce_group([(e1[d][:], e2L[d]) for d in range(NDT)], pos_main, "main")
        if r_self > 0:
            bx1 = [boxsum(e1[d][:], r_self, f"s1_{d}") for d in range(NDT)]
            bx2 = [boxsum(e2L[d], r_self, f"s2_{d}") for d in range(NDT)]
            reduce_group([(e1[d][:], bx1[d]) for d in range(NDT)]
                         + [(e2L[d], bx2[d]) for d in range(NDT)], pos_self, "self")
        else:
            zr = rows.tile([1, L], dt.float32, tag="zr")
            nc.vector.memset(zr[:], 0.0)
            nc.sync.dma_start(out=pos_self[:].rearrange("(one n) -> one n", one=1), in_=zr[:])
        if r_tgt > 0:
            bxt = [boxsum(e2L[d], r_tgt, f"t_{d}") for d in range(NDT)]
            reduce_group([(e1[d][:], bxt[d]) for d in range(NDT)], pos_tgt, "tgt")
        else:
            zr2 = rows.tile([1, L], dt.float32, tag="zr2")
            nc.vector.memset(zr2[:], 0.0)
            nc.sync.dma_start(out=pos_tgt[:].rearrange("(one n) -> one n", one=1), in_=zr2[:])

        # diag bias column layout: [128, 8] with diag[p, a] = pos_main[a*128+p]
        neg_diag = const.tile([128, 8], dt.float32)
        nc.sync.dma_start(out=neg_diag[:], in_=pos_main[:].rearrange("(a p) -> p a", p=128))
        nc.vector.tensor_scalar_mul(neg_diag[:], neg_diag[:], -1.0)

        # ---- rest of e2 projection (chunks 1..7) ------------------------
        for c in range(1, B):
            project(f2t, c * L, L, e2, c * L)

        # ---- sim slab + streaming exp-sum -------------------------------
        ncg = N // CW                           # column groups per row tile
        stot = const.tile([128, 8 * ncg], dt.float32)
        for m in range(8):                      # 128-token row tiles
            for c in range(ncg):                # CW-wide column groups
                ps = mmp.tile([128, CW], dt.float32, tag="mm")
                for d_ in range(NDT):           # weight-stationary inner order
                    for q in range(CW // 512):
                        nc.tensor.matmul(
                            ps[:, q * 512:(q + 1) * 512],
                            e1[d_][:, m * 128:(m + 1) * 128],
                            e2[d_][:, c * CW + q * 512: c * CW + (q + 1) * 512],
                            start=(d_ == 0), stop=(d_ == NDT - 1))
                ex = esc.tile([128, CW], dt.bfloat16, tag="ex")
                nc.scalar.activation(ex[:], ps[:], AF.Exp,
                                     bias=neg_diag[:, m:m + 1], scale=1.0,
                                     accum_out=stot[:, m * ncg + c: m * ncg + c + 1])
        nc.sync.dma_start(out=s_out[:], in_=stot[:])

    nc.compile()
    return nc


def kernel(feature1, feature2, W, b, positive_range_self, positive_range_tgt):
    r_self = int(np.asarray(positive_range_self))
    r_tgt = int(np.asarray(positive_range_tgt))
    assert 0 <= r_self <= PAD and 0 <= r_tgt <= PAD

    key = (r_self, r_tgt)
    if key not in _module_cache:
        _module_cache[key] = _build(r_self, r_tgt)
    nc = _module_cache[key]

    in_maps = _make_in_maps(feature1, feature2, W, b)
    res = bass_utils.run_bass_kernel_spmd(nc, in_maps, list(range(N_CORES)))

    # ---- host combine (fp64) ---------------------------------------------
    j = np.arange(L)
    loss_terms = []
    for i in range(N_CORES):
        r = res.results[i]
        # S groups: stot[p, m*ncg + c]; token j = m*128 + p; sum over c groups
        ncg = N // CW
        S = r["s_out"].astype(np.float64).reshape(128, 8, ncg).sum(axis=2)
        S = S.T.reshape(L)                                   # token j at [j%128, j//128]
        t = np.log(S) - np.log(float(N))                     # negative - diag (diag cancels)
        if r_self > 0:
            cnt = np.minimum(L - 1, j + r_self) - np.maximum(0, j - r_self) + 1.0
            t -= r["pos_self"].astype(np.float64) / cnt
        if r_tgt > 0:
            cnt = np.minimum(L - 1, j + r_tgt) - np.maximum(0, j - r_tgt) + 1.0
            t -= r["pos_tgt"].astype(np.float64) / cnt
        loss_terms.append(t)
    loss = np.mean(np.concatenate(loss_terms))
    return np.float32(loss)


def _make_in_maps(feature1, feature2, W, b):
    bf16 = ml_dtypes.bfloat16
    f1 = np.asarray(feature1, dtype=np.float32)
    f2 = np.asarray(feature2, dtype=np.float32)
    Wr = np.ascontiguousarray(np.asarray(W, dtype=np.float32).astype(bf16))
    bv = np.ascontiguousarray(np.asarray(b, dtype=np.float32))
    f2t_full = f2.reshape(N, DIN).T.astype(bf16)             # [DIN, N]
    in_maps = []
    for i in range(N_CORES):
        f1t_i = np.ascontiguousarray(f1[i].T.astype(bf16))   # [DIN, L]
        f2t_rot = np.ascontiguousarray(np.roll(f2t_full, -i * L, axis=1))
        in_maps.append({"f1t": f1t_i, "f2t": f2t_rot, "w_in": Wr, "b_in": bv})
    return in_maps



# revision 4
# speedup vs baseline: 1.7413x; 1.7413x over previous
"""Contrastive-learning NCE loss on 8 trn2 NeuronCores (Bass/Tile).

Problem (hardcoded shapes): B=8, L=1024, D_in=512, D_feat=256, N=B*L=8192.
  emb_k = relu(feature_k @ W + b)                     [B, L, Df]
  positive = <e1,e2> + banded_diag_mean terms         [N]
  negative = logsumexp(e1 @ e2.T, axis=-1) - log(N)   [N]
  loss = mean(-positive + negative)

Sharding: token dim N split across 8 cores = one batch row each (L == N/8).
Each core computes its [1024, 8192] slab of the similarity matrix against the
full emb_2 (recomputed locally from full feature2). The host rotates feature2
per core so the core's own batch always sits at columns 0:1023 -> the SPMD
program is core-index free.

v2 design (vs the bf16 baseline at ~160-187us):
  * All matmuls in fp8e4m3 with DoubleRow perf mode: one MM contracts K=256
    (two interleaved 128-k-tiles), so the sim slab halves to 128 MMs/core and
    the projection runs at 2x the bf16 rate. HW-validated: same ~216-225ns/MM
    spacing as bf16 at 512 output cols, rel err ~1.5e-4.
  * logsumexp shift is a CONSTANT -64 (exact for any shift; max sim entry is
    ~120 so exp(sim-64) stays well inside fp32), removing the per-row-diag
    bias and its DRAM-roundtrip transpose from the critical path.
  * exp runs in-place on the PSUM tile (ScalarE is closest to PSUM) with the
    fused row-accumulator; host adds 64 + log(S). ACT is the roofline engine:
    32 groups x (~1.97us ACTIVATE + ~0.28us accum-read) ~= 72us/core.
  * Software-pipelined schedule keeps ACT streaming from ~7us: sim blocks
    (PE+ACT) overlap the remaining e2 projection chunks (PE+DVE) and the
    banded-term work (DVE + a few tiny PE reduces), which is slotted into
    engine-idle windows of specific blocks.
  * Banded positive terms in bf16 from dedicated bf16 copies of the own-batch
    embeddings; box-filter of e2 is reused for the tgt term when
    r_tgt == r_self.
"""

import numpy as np
import ml_dtypes
from contextlib import ExitStack

import concourse.bass as bass
import concourse.tile as tile
from concourse import bacc, mybir
from concourse import bass_utils

dt = mybir.dt
AF = mybir.ActivationFunctionType
ALU = mybir.AluOpType
PM = mybir.MatmulPerfMode

N_CORES = 8
B, L, DIN, DF = 8, 1024, 512, 256
N = B * L
KO = DIN // 128     # 4 k-tiles of the projection contraction
NDT = DF // 128     # 2 d-tiles of the embedding dim
PAD = 4             # box-filter padding (max supported positive_range)
CW = 2048           # sim-phase column group (one PSUM tile / one ACT)
NCG = N // CW       # 4 sim blocks
SHIFT = 64.0        # constant logsumexp shift

_module_cache = {}


def _box_terms(w: int):
    """Decompose window width w (odd, <= 2*PAD+1) into power-of-2 segments."""
    terms, off = [], 0
    for p in (8, 4, 2, 1):
        if w >= p:
            terms.append((p, off))
            off += p
            w -= p
    assert w == 0
    return terms


def _build(r_self: int, r_tgt: int):
    nc = bacc.Bacc("TRN2", target_bir_lowering=False, debug=False, num_devices=N_CORES)

    f1t = nc.dram_tensor("f1t", [DIN, L], dt.float8e4, kind="ExternalInput").ap()
    f2t = nc.dram_tensor("f2t", [DIN, N], dt.float8e4, kind="ExternalInput").ap()
    w_in = nc.dram_tensor("w_in", [DIN, DF], dt.float8e4, kind="ExternalInput").ap()
    b_in = nc.dram_tensor("b_in", [DF], dt.float32, kind="ExternalInput").ap()

    pos_main = nc.dram_tensor("pos_main", [L], dt.float32, kind="ExternalOutput").ap()
    pos_self = nc.dram_tensor("pos_self", [L], dt.float32, kind="ExternalOutput").ap()
    pos_tgt = nc.dram_tensor("pos_tgt", [L], dt.float32, kind="ExternalOutput").ap()
    s_out = nc.dram_tensor("s_out", [128, 8 * NCG], dt.float32, kind="ExternalOutput").ap()

    with tile.TileContext(nc) as tc, ExitStack() as ctx:
        const = ctx.enter_context(tc.tile_pool(name="const", bufs=1))
        stage = ctx.enter_context(tc.tile_pool(name="stage", bufs=3))
        emb = ctx.enter_context(tc.tile_pool(name="emb", bufs=1))
        band = ctx.enter_context(tc.tile_pool(name="band", bufs=1))
        prodp = ctx.enter_context(tc.tile_pool(name="prodp", bufs=8))
        rows = ctx.enter_context(tc.tile_pool(name="rows", bufs=1))
        mmp = ctx.enter_context(tc.tile_pool(name="mmp", bufs=2, space="PSUM"))

        # ---- constants -------------------------------------------------
        wt = const.tile([128, KO, DF], dt.float8e4)    # W[ko*128+p, d] at [p, ko, d]
        nc.sync.dma_start(out=wt[:], in_=w_in[:].rearrange("(ko p) d -> p ko d", p=128))
        b_col = const.tile([128, NDT], dt.float32)     # bias per (d%128, dtile)
        nc.sync.dma_start(out=b_col[:], in_=b_in[:].rearrange("(d p) -> p d", p=128))
        ones_f = const.tile([128, 1], dt.float32)
        nc.vector.memset(ones_f[:], 1.0)
        ones = const.tile([128, 1], dt.bfloat16)
        nc.vector.tensor_copy(ones[:], ones_f[:])
        neg_shift = const.tile([128, 1], dt.float32)
        nc.vector.memset(neg_shift[:], -SHIFT)
        warm = const.tile([128, 1], dt.float32)
        # dummy exp so the ACT exp-table load happens during the DMA/proj phase
        nc.scalar.activation(warm[:], ones_f[:], AF.Exp, bias=neg_shift[:], scale=1.0)

        # ---- embeddings ------------------------------------------------
        # fp8 k-tile-major layout for DoubleRow: element (p, d, t) = emb[d*128+p, t]
        e1q = emb.tile([128, NDT, L], dt.float8e4, name="e1q", tag="e1q")
        e2q = emb.tile([128, NDT, N], dt.float8e4, name="e2q", tag="e2q")
        # bf16 copies (own batch only) for the banded terms
        e1b = emb.tile([128, NDT, L], dt.bfloat16, name="e1b", tag="e1b")
        e2b = emb.tile([128, NDT, L], dt.bfloat16, name="e2b", tag="e2b")

        stot = const.tile([128, 8 * NCG], dt.float32)

        def project(src_ap, col0, q_dst, q_col0, b_dst=None):
            """relu-project fT cols [col0, col0+1024) into q_dst (fp8) and
            optionally b_dst (bf16). 8 PE DoubleRow MMs + NDT (or 2*NDT) DVE TS."""
            nco = 1024
            fst = stage.tile([128, KO, nco], dt.float8e4, tag="fstage")
            nc.sync.dma_start(
                out=fst[:],
                in_=src_ap[:, col0:col0 + nco].rearrange("(ko p) n -> p ko n", p=128))
            for d in range(NDT):
                ps = mmp.tile([128, nco], dt.float32, tag="mm", name=f"pj_{col0}_{d}")
                for kk in range(KO // 2):
                    for h in range(nco // 512):
                        nc.tensor.matmul(
                            ps[:, h * 512:(h + 1) * 512],
                            wt[:, 2 * kk:2 * kk + 2, d * 128:(d + 1) * 128],
                            fst[:, 2 * kk:2 * kk + 2, h * 512:(h + 1) * 512],
                            start=(kk == 0), stop=(kk == KO // 2 - 1),
                            perf_mode=PM.DoubleRow)
                nc.vector.tensor_scalar(
                    q_dst[:, d, q_col0:q_col0 + nco], ps[:],
                    b_col[:, d:d + 1], 0.0, ALU.add, ALU.max)
                if b_dst is not None:
                    nc.vector.tensor_scalar(
                        b_dst[:, d, 0:nco], ps[:],
                        b_col[:, d:d + 1], 0.0, ALU.add, ALU.max)

        def boxsum(src_view, r, tag):
            """[128, L] view: out[:, j] = sum_{|d|<=r} src[:, j+d] (clipped)."""
            wdt = 2 * r + 1
            pb = band.tile([128, L + 2 * PAD], dt.bfloat16, name=f"pb_{tag}",
                           tag="pb", bufs=2)
            nc.vector.memzero(pb[:])
            nc.vector.tensor_copy(pb[:, PAD:PAD + L], src_view)
            s = {1: pb}
            for p in (2, 4, 8):
                if wdt >= p:
                    sp = band.tile([128, L + 2 * PAD], dt.bfloat16, name=f"s{p}_{tag}",
                                   tag=f"s{p}")
                    h = p // 2
                    n_valid = L + 2 * PAD - p + 1
                    nc.vector.tensor_tensor(
                        sp[:, :n_valid], s[h][:, :n_valid], s[h][:, h:h + n_valid], ALU.add)
                    s[p] = sp
            terms = _box_terms(wdt)
            t0 = PAD - r
            if len(terms) == 1:
                p0, o0 = terms[0]
                return s[p0][:, t0 + o0: t0 + o0 + L]
            acc = band.tile([128, L], dt.bfloat16, name=f"box_{tag}", tag="box", bufs=6)
            p0, o0 = terms[0]
            p1, o1 = terms[1]
            nc.vector.tensor_tensor(acc[:], s[p0][:, t0 + o0: t0 + o0 + L],
                                    s[p1][:, t0 + o1: t0 + o1 + L], ALU.add)
            for p, o in terms[2:]:
                nc.vector.tensor_tensor(acc[:], acc[:], s[p][:, t0 + o: t0 + o + L], ALU.add)
            return acc[:]

        def sim_group(bk, m):
            """One [128 rows, CW cols] sim slab chunk + in-place exp/accum."""
            ps = mmp.tile([128, CW], dt.float32, tag="mm", name=f"sim_{bk}_{m}")
            for q in range(CW // 512):
                nc.tensor.matmul(
                    ps[:, q * 512:(q + 1) * 512],
                    e1q[:, :, m * 128:(m + 1) * 128],
                    e2q[:, :, bk * CW + q * 512: bk * CW + (q + 1) * 512],
                    start=True, stop=True, perf_mode=PM.DoubleRow)
            nc.scalar.activation(ps[:], ps[:], AF.Exp,
                                 bias=neg_shift[:], scale=1.0,
                                 accum_out=stot[:, m * NCG + bk: m * NCG + bk + 1])

        def sim_block(bk, interleave=None):
            for m in range(8):
                sim_group(bk, m)
                if interleave and m in interleave:
                    interleave[m]()

        # ---- banded-term pieces (issued at chosen schedule points) ------
        e1v = [e1b[:, d, :] for d in range(NDT)]
        e2v = [e2b[:, d, :] for d in range(NDT)]

        def make_prods(pairs, tag):
            prods = []
            for gi, (a_view, b_view) in enumerate(pairs):
                prod = prodp.tile([128, L], dt.bfloat16, tag=f"prod_{tag}_{gi}", bufs=1)
                nc.vector.tensor_tensor(prod[:], a_view, b_view, ALU.mult)
                prods.append(prod)
            return prods

        def make_reduce_mm(prods, tag):
            # matmul out must fit one PSUM bank (512 fp32) -> two 512 halves
            rp = mmp.tile([1, L], dt.float32, tag="mm", name=f"rp_{tag}")
            for h in range(L // 512):
                for gi, prod in enumerate(prods):
                    nc.tensor.matmul(rp[:, h * 512:(h + 1) * 512], ones[:],
                                     prod[:, h * 512:(h + 1) * 512],
                                     start=(gi == 0), stop=(gi == len(prods) - 1))
            return rp

        def finish_row(rp, out_dram, tag):
            row = rows.tile([1, L], dt.float32, tag=f"row_{tag}")
            nc.vector.tensor_copy(row[:], rp[:])
            nc.sync.dma_start(out=out_dram[:].rearrange("(one n) -> one n", one=1),
                              in_=row[:])

        def zero_out(out_dram, tag):
            zr = rows.tile([1, L], dt.float32, tag=f"zr_{tag}")
            nc.vector.memset(zr[:], 0.0)
            nc.sync.dma_start(out=out_dram[:].rearrange("(one n) -> one n", one=1),
                              in_=zr[:])

        # ---- schedule ---------------------------------------------------
        project(f1t, 0, e1q, 0, e1b)           # own batch features 1
        project(f2t, 0, e2q, 0, e2b)           # own batch (rotated cols 0:L)
        project(f2t, L, e2q, L)                # chunk 1

        sim_block(0)
        project(f2t, 2 * L, e2q, 2 * L)
        project(f2t, 3 * L, e2q, 3 * L)

        # boxsums: DVE-only, runs in block-1's ACT window
        bx1 = [boxsum(e1v[d], r_self, f"s1_{d}") for d in range(NDT)] if r_self else None
        bx2 = [boxsum(e2v[d], r_self, f"s2_{d}") for d in range(NDT)] if r_self else None
        if r_tgt and r_tgt != r_self:
            bxt = [boxsum(e2v[d], r_tgt, f"t_{d}") for d in range(NDT)]
        else:
            bxt = bx2 if r_tgt else None

        sim_block(1)
        project(f2t, 4 * L, e2q, 4 * L)
        project(f2t, 5 * L, e2q, 5 * L)

        # elementwise products: DVE-only, runs in block-2's early ACT window
        pr_main = make_prods(list(zip(e1v, e2v)), "main")
        pr_self = make_prods(list(zip(e1v, bx1)) + list(zip(e2v, bx2)), "self") \
            if r_self else None
        pr_tgt = make_prods(list(zip(e1v, bxt)), "tgt") if r_tgt else None

        # tiny PE reduce-MMs interleaved into block 2 (prods are ready by then);
        # their row-copies + DMAs follow on the DVE before the c6/c7 consumers
        rps = {}
        ilv = {1: lambda: rps.__setitem__("main", make_reduce_mm(pr_main, "main"))}
        if r_self:
            ilv[3] = lambda: rps.__setitem__("self", make_reduce_mm(pr_self, "self"))
        if r_tgt:
            ilv[5] = lambda: rps.__setitem__("tgt", make_reduce_mm(pr_tgt, "tgt"))
        sim_block(2, interleave=ilv)

        finish_row(rps["main"], pos_main, "main")
        if r_self:
            finish_row(rps["self"], pos_self, "self")
        else:
            zero_out(pos_self, "self")
        if r_tgt:
            finish_row(rps["tgt"], pos_tgt, "tgt")
        else:
            zero_out(pos_tgt, "tgt")

        project(f2t, 6 * L, e2q, 6 * L)
        project(f2t, 7 * L, e2q, 7 * L)
        sim_block(3)

        nc.sync.dma_start(out=s_out[:], in_=stot[:])

    nc.compile()
    return nc


def kernel(feature1, feature2, W, b, positive_range_self, positive_range_tgt):
    r_self = int(np.asarray(positive_range_self))
    r_tgt = int(np.asarray(positive_range_tgt))
    assert 0 <= r_self <= PAD and 0 <= r_tgt <= PAD

    key = (r_self, r_tgt)
    if key not in _module_cache:
        _module_cache[key] = _build(r_self, r_tgt)
    nc = _module_cache[key]

    in_maps = _make_in_maps(feature1, feature2, W, b)
    res = bass_utils.run_bass_kernel_spmd(nc, in_maps, list(range(N_CORES)))

    # ---- host combine (fp64) ---------------------------------------------
    j = np.arange(L)
    loss_terms = []
    for i in range(N_CORES):
        r = res.results[i]
        # stot[p, m*NCG + bk]; token j = m*128 + p; sum over the NCG blocks
        S = r["s_out"].astype(np.float64).reshape(128, 8, NCG).sum(axis=2)
        S = S.T.reshape(L)                                   # token j at [j%128, j//128]
        t = SHIFT + np.log(S) - np.log(float(N))             # negative_j
        t -= r["pos_main"].astype(np.float64)
        if r_self > 0:
            cnt = np.minimum(L - 1, j + r_self) - np.maximum(0, j - r_self) + 1.0
            t -= r["pos_self"].astype(np.float64) / cnt
        if r_tgt > 0:
            cnt = np.minimum(L - 1, j + r_tgt) - np.maximum(0, j - r_tgt) + 1.0
            t -= r["pos_tgt"].astype(np.float64) / cnt
        loss_terms.append(t)
    loss = np.mean(np.concatenate(loss_terms))
    return np.float32(loss)


def _make_in_maps(feature1, feature2, W, b):
    fp8 = ml_dtypes.float8_e4m3
    f1 = np.asarray(feature1, dtype=np.float32)
    f2 = np.asarray(feature2, dtype=np.float32)
    Wr = np.ascontiguousarray(np.asarray(W, dtype=np.float32).astype(fp8))
    bv = np.ascontiguousarray(np.asarray(b, dtype=np.float32))
    f2t_full = f2.reshape(N, DIN).T.astype(fp8)              # [DIN, N]
    in_maps = []
    for i in range(N_CORES):
        f1t_i = np.ascontiguousarray(f1[i].T.astype(fp8))    # [DIN, L]
        f2t_rot = np.ascontiguousarray(np.roll(f2t_full, -i * L, axis=1))
        in_maps.append({"f1t": f1t_i, "f2t": f2t_rot, "w_in": Wr, "b_in": bv})
    return in_maps


# revision 6
# speedup vs baseline: 1.8417x; 1.0577x over previous
"""Contrastive-learning NCE loss on 8 trn2 NeuronCores (Bass/Tile).

Problem (hardcoded shapes): B=8, L=1024, D_in=512, D_feat=256, N=B*L=8192.
  emb_k = relu(feature_k @ W + b)                     [B, L, Df]
  positive = <e1,e2> + banded_diag_mean terms         [N]
  negative = logsumexp(e1 @ e2.T, axis=-1) - log(N)   [N]
  loss = mean(-positive + negative)

Sharding: token dim N split across 8 cores = one batch row each (L == N/8).
Each core computes its [1024, 8192] slab of the similarity matrix against the
full emb_2 (recomputed locally from full feature2). The host rotates feature2
per core so the core's own batch always sits at columns 0:1023 -> the SPMD
program is core-index free.

v3 design (baseline bf16: ~160-187us; v2 fp8: ~126us):
  * All matmuls fp8e4m3 + DoubleRow (K=256 per MM, 2x bf16 rate; validated
    ~220ns/MM at 512 cols, rel err ~1.5e-4).
  * Constant logsumexp shift (-64): exact for any shift, max sim ~120 keeps
    exp(sim-64) in fp32. exp runs in-place on PSUM with the fused row
    accumulator; host adds 64 + log(S). ACT (ScalarE) is the roofline:
    32 groups x (~1.97us + ~0.29us accum-read) ~= 72us/core.
  * The whole kernel is one software pipeline paced by ACT. The PSUM "mm"
    ring (2 x [128,2048] = all 8 banks) is shared by sim groups and
    projection tiles; projection d-tiles are interleaved INSIDE the sim
    m-loop so every ring slot's consumer (ACT exp or one DVE tensor_scalar)
    has ~2 ACT-groups of slack -- no block-boundary stalls, PE never idles
    >2us (HAM stays warm).
  * bf16 copies of the own-batch embeddings are made from the fp8 SBUF
    tiles (not from PSUM), so each projection PSUM tile has exactly one
    consumer. The copies live in padded [128, NDT, L+2*PAD] tiles (borders
    zeroed once) so the banded box-filter skips its staging step.
  * Banded-term DVE work (copies, box-filters, products) is queued as
    closures and drained one-per-injection-point between projection
    consumers, keeping the DVE from ever blocking the pipeline. The tiny
    PE reduce-MMs for pos_main/self/tgt slot into block 3 (where no
    projection remains), with row evacuation + DMA interleaved after.
"""

import numpy as np
import ml_dtypes
from collections import deque
from contextlib import ExitStack

import concourse.bass as bass
import concourse.tile as tile
from concourse import bacc, mybir
from concourse import bass_utils

dt = mybir.dt
AF = mybir.ActivationFunctionType
ALU = mybir.AluOpType
PM = mybir.MatmulPerfMode

N_CORES = 8
B, L, DIN, DF = 8, 1024, 512, 256
N = B * L
KO = DIN // 128     # 4 k-tiles of the projection contraction
NDT = DF // 128     # 2 d-tiles of the embedding dim
PAD = 4             # box-filter padding (max supported positive_range)
LP = L + 2 * PAD
CW = 2048           # sim-phase column group (one PSUM tile / one ACT)
NCG = N // CW       # 4 sim blocks
SHIFT = 64.0        # constant logsumexp shift

_module_cache = {}


def _box_terms(w: int):
    """Decompose window width w (odd, <= 2*PAD+1) into power-of-2 segments."""
    terms, off = [], 0
    for p in (8, 4, 2, 1):
        if w >= p:
            terms.append((p, off))
            off += p
            w -= p
    assert w == 0
    return terms


def _build(r_self: int, r_tgt: int):
    nc = bacc.Bacc("TRN2", target_bir_lowering=False, debug=False, num_devices=N_CORES)

    f1t = nc.dram_tensor("f1t", [DIN, L], dt.float8e4, kind="ExternalInput").ap()
    f2t = nc.dram_tensor("f2t", [DIN, N], dt.float8e4, kind="ExternalInput").ap()
    w_in = nc.dram_tensor("w_in", [DIN, DF], dt.float8e4, kind="ExternalInput").ap()
    b_in = nc.dram_tensor("b_in", [DF], dt.float32, kind="ExternalInput").ap()

    pos_main = nc.dram_tensor("pos_main", [L], dt.float32, kind="ExternalOutput").ap()
    pos_self = nc.dram_tensor("pos_self", [L], dt.float32, kind="ExternalOutput").ap()
    pos_tgt = nc.dram_tensor("pos_tgt", [L], dt.float32, kind="ExternalOutput").ap()
    s_out = nc.dram_tensor("s_out", [128, 8 * NCG], dt.float32, kind="ExternalOutput").ap()

    with tile.TileContext(nc) as tc, ExitStack() as ctx:
        const = ctx.enter_context(tc.tile_pool(name="const", bufs=1))
        stage = ctx.enter_context(tc.tile_pool(name="stage", bufs=3))
        emb = ctx.enter_context(tc.tile_pool(name="emb", bufs=1))
        band = ctx.enter_context(tc.tile_pool(name="band", bufs=1))
        prodp = ctx.enter_context(tc.tile_pool(name="prodp", bufs=8))
        rows = ctx.enter_context(tc.tile_pool(name="rows", bufs=1))
        mmp = ctx.enter_context(tc.tile_pool(name="mmp", bufs=2, space="PSUM"))

        # ---- constants -------------------------------------------------
        wt = const.tile([128, KO, DF], dt.float8e4)    # W[ko*128+p, d] at [p, ko, d]
        nc.sync.dma_start(out=wt[:], in_=w_in[:].rearrange("(ko p) d -> p ko d", p=128))
        b_col = const.tile([128, NDT], dt.float32)     # bias per (d%128, dtile)
        nc.sync.dma_start(out=b_col[:], in_=b_in[:].rearrange("(d p) -> p d", p=128))
        ones_f = const.tile([128, 1], dt.float32)
        nc.vector.memset(ones_f[:], 1.0)
        ones = const.tile([128, 1], dt.bfloat16)
        nc.vector.tensor_copy(ones[:], ones_f[:])
        neg_shift = const.tile([128, 1], dt.float32)
        nc.vector.memset(neg_shift[:], -SHIFT)
        warm = const.tile([128, 1], dt.float32)
        # dummy exp so the ACT exp-table load happens during the DMA/proj phase
        nc.scalar.activation(warm[:], ones_f[:], AF.Exp, bias=neg_shift[:], scale=1.0)

        # ---- embeddings ------------------------------------------------
        # fp8 k-tile-major layout for DoubleRow: element (p, d, t) = emb[d*128+p, t]
        e1q = emb.tile([128, NDT, L], dt.float8e4, name="e1q", tag="e1q")
        e2q = emb.tile([128, NDT, N], dt.float8e4, name="e2q", tag="e2q")
        # bf16 copies (own batch, PAD-padded for the box filter; borders zeroed)
        e1b = emb.tile([128, NDT, LP], dt.bfloat16, name="e1b", tag="e1b")
        e2b = emb.tile([128, NDT, LP], dt.bfloat16, name="e2b", tag="e2b")
        nc.vector.memzero(e1b[:])
        nc.vector.memzero(e2b[:])

        stot = const.tile([128, 8 * NCG], dt.float32)

        # ---- projection pieces -----------------------------------------
        def pj_mm(src_ap, col0, d, tag):
            """PE half of a projection d-tile: DMA stage (once per chunk) +
            2 accumulating DoubleRow MMs per 512-col half. Returns PSUM tile."""
            nco = 1024
            if d == 0:
                fst = stage.tile([128, KO, nco], dt.float8e4, tag="fstage",
                                 name=f"fst_{tag}")
                nc.sync.dma_start(
                    out=fst[:],
                    in_=src_ap[:, col0:col0 + nco].rearrange("(ko p) n -> p ko n", p=128))
                pj_mm.fst = fst
            fst = pj_mm.fst
            ps = mmp.tile([128, nco], dt.float32, tag="mm", name=f"pj_{tag}_{d}")
            for kk in range(KO // 2):
                for h in range(nco // 512):
                    nc.tensor.matmul(
                        ps[:, h * 512:(h + 1) * 512],
                        wt[:, 2 * kk:2 * kk + 2, d * 128:(d + 1) * 128],
                        fst[:, 2 * kk:2 * kk + 2, h * 512:(h + 1) * 512],
                        start=(kk == 0), stop=(kk == KO // 2 - 1),
                        perf_mode=PM.DoubleRow)
            return ps

        def pj_cons(ps, q_dst, d, q_col0):
            """DVE half: relu(ps + b) -> fp8 destination (single PSUM consumer)."""
            nc.vector.tensor_scalar(
                q_dst[:, d, q_col0:q_col0 + 1024], ps[:],
                b_col[:, d:d + 1], 0.0, ALU.add, ALU.max)

        def project(src_ap, col0, q_dst, q_col0):
            for d in range(NDT):
                ps = pj_mm(src_ap, col0, d, f"c{col0}")
                pj_cons(ps, q_dst, d, q_col0)

        # ---- sim group --------------------------------------------------
        def sim_group(bk, m):
            ps = mmp.tile([128, CW], dt.float32, tag="mm", name=f"sim_{bk}_{m}")
            for q in range(CW // 512):
                nc.tensor.matmul(
                    ps[:, q * 512:(q + 1) * 512],
                    e1q[:, :, m * 128:(m + 1) * 128],
                    e2q[:, :, bk * CW + q * 512: bk * CW + (q + 1) * 512],
                    start=True, stop=True, perf_mode=PM.DoubleRow)
            nc.scalar.activation(ps[:], ps[:], AF.Exp,
                                 bias=neg_shift[:], scale=1.0,
                                 accum_out=stot[:, m * NCG + bk: m * NCG + bk + 1])

        # ---- banded-term closures (drained between pipeline slots) ------
        boxes = {}
        prods = {}

        def mk_copy(dst, src_q, d):
            def f():
                nc.vector.tensor_copy(dst[:, d, PAD:PAD + L], src_q[:, d, 0:L])
            return f

        def mk_boxsum(key, src, d, r):
            """src: padded [128, NDT, LP] tile; result view stored in boxes."""
            def f():
                wdt = 2 * r + 1
                s = {1: src[:, d, :]}
                for p in (2, 4, 8):
                    if wdt >= p:
                        sp = band.tile([128, LP], dt.bfloat16, name=f"s{p}_{key}",
                                       tag=f"s{p}")
                        h = p // 2
                        nv = LP - p + 1
                        nc.vector.tensor_tensor(sp[:, :nv], s[h][:, :nv],
                                                s[h][:, h:h + nv], ALU.add)
                        s[p] = sp
                terms = _box_terms(wdt)
                t0 = PAD - r
                if len(terms) == 1:
                    p0, o0 = terms[0]
                    boxes[key] = s[p0][:, t0 + o0: t0 + o0 + L]
                    return
                acc = band.tile([128, L], dt.bfloat16, name=f"box_{key}",
                                tag="box", bufs=6)
                p0, o0 = terms[0]
                p1, o1 = terms[1]
                nc.vector.tensor_tensor(acc[:], s[p0][:, t0 + o0: t0 + o0 + L],
                                        s[p1][:, t0 + o1: t0 + o1 + L], ALU.add)
                for p, o in terms[2:]:
                    nc.vector.tensor_tensor(acc[:], acc[:],
                                            s[p][:, t0 + o: t0 + o + L], ALU.add)
                boxes[key] = acc[:]
            return f

        def mk_prods(key, pairs_fn):
            def f():
                out = []
                for gi, (a_view, b_view) in enumerate(pairs_fn()):
                    prod = prodp.tile([128, L], dt.bfloat16,
                                      tag=f"prod_{key}_{gi}", bufs=1)
                    nc.vector.tensor_tensor(prod[:], a_view, b_view, ALU.mult)
                    out.append(prod)
                prods[key] = out
            return f

        e1v = [e1b[:, d, PAD:PAD + L] for d in range(NDT)]
        e2v = [e2b[:, d, PAD:PAD + L] for d in range(NDT)]

        dq = deque()
        dq.append(mk_copy(e1b, e1q, 0))
        dq.append(mk_copy(e1b, e1q, 1))
        dq.append(mk_copy(e2b, e2q, 0))
        dq.append(mk_copy(e2b, e2q, 1))
        dq.append(mk_prods("main", lambda: list(zip(e1v, e2v))))
        if r_self:
            for d in range(NDT):
                dq.append(mk_boxsum(("bx1", d), e1b, d, r_self))
            for d in range(NDT):
                dq.append(mk_boxsum(("bx2", d), e2b, d, r_self))
            dq.append(mk_prods("self", lambda: [(e1v[d], boxes[("bx1", d)]) for d in range(NDT)]
                               + [(e2v[d], boxes[("bx2", d)]) for d in range(NDT)]))
        if r_tgt:
            if r_tgt != r_self:
                for d in range(NDT):
                    dq.append(mk_boxsum(("bxt", d), e2b, d, r_tgt))
                tkey = "bxt"
            else:
                tkey = "bx2"
            dq.append(mk_prods("tgt", lambda: [(e1v[d], boxes[(tkey, d)]) for d in range(NDT)]))

        def drain(k=1):
            for _ in range(k):
                if dq:
                    dq.popleft()()

        # ---- pos reduce-MMs + row evacuation (block 3) -------------------
        rps = {}

        def mk_reduce_mm(key):
            def f():
                rp = mmp.tile([1, L], dt.float32, tag="mm", name=f"rp_{key}")
                pr = prods[key]
                for h in range(L // 512):
                    for gi, prod in enumerate(pr):
                        nc.tensor.matmul(rp[:, h * 512:(h + 1) * 512], ones[:],
                                         prod[:, h * 512:(h + 1) * 512],
                                         start=(gi == 0), stop=(gi == len(pr) - 1))
                rps[key] = rp
            return f

        def mk_row(key, out_dram):
            def f():
                row = rows.tile([1, L], dt.float32, tag=f"row_{key}")
                nc.vector.tensor_copy(row[:], rps[key][:])
                nc.sync.dma_start(out=out_dram[:].rearrange("(one n) -> one n", one=1),
                                  in_=row[:])
            return f

        def zero_out(out_dram, tag):
            zr = rows.tile([1, L], dt.float32, tag=f"zr_{tag}")
            nc.vector.memset(zr[:], 0.0)
            nc.sync.dma_start(out=out_dram[:].rearrange("(one n) -> one n", one=1),
                              in_=zr[:])

        red_list = [("main", pos_main)]
        if r_self:
            red_list.append(("self", pos_self))
        else:
            zero_out(pos_self, "self")
        if r_tgt:
            red_list.append(("tgt", pos_tgt))
        else:
            zero_out(pos_tgt, "tgt")

        # ---- schedule ---------------------------------------------------
        # prologue: project f1 + e2 chunks 0,1 (lean: only the fp8 consumers)
        project(f1t, 0, e1q, 0)
        project(f2t, 0, e2q, 0)
        project(f2t, L, e2q, L)

        # blocks: sim m-loop with projection d-tiles (blocks 0-2) or the pos
        # reduce-MMs and row evacuation (block 3) slotted inside
        for bk in range(NCG):
            mm_slots = {}   # after-m -> PE closure
            dv_slots = {}   # after-m -> DVE closure
            if bk < 3:
                c0, c1 = 2 * bk + 2, 2 * bk + 3
                for idx, (cc, d) in enumerate([(c0, 0), (c0, 1), (c1, 0), (c1, 1)]):
                    m_at = (2, 4, 6, 7)[idx]

                    def mk(cc=cc, d=d):
                        def f():
                            ps = pj_mm(f2t, cc * L, d, f"c{cc}")
                            pj_cons(ps, e2q, d, cc * L)
                            drain(1)
                        return f
                    mm_slots[m_at] = mk()
            else:
                for idx, (key, out_dram) in enumerate(red_list):
                    mm_slots[2 * idx + 1] = mk_reduce_mm(key)
                    dv_slots[2 * idx + 3] = mk_row(key, out_dram)
            for m in range(8):
                sim_group(bk, m)
                if m in mm_slots:
                    mm_slots[m]()
                if m in dv_slots:
                    dv_slots[m]()
        drain(len(dq))   # leftover banded work (normally empty by block 2)

        nc.sync.dma_start(out=s_out[:], in_=stot[:])

    nc.compile()
    return nc


def kernel(feature1, feature2, W, b, positive_range_self, positive_range_tgt):
    r_self = int(np.asarray(positive_range_self))
    r_tgt = int(np.asarray(positive_range_tgt))
    assert 0 <= r_self <= PAD and 0 <= r_tgt <= PAD

    key = (r_self, r_tgt)
    if key not in _module_cache:
        _module_cache[key] = _build(r_self, r_tgt)
    nc = _module_cache[key]

    in_maps = _make_in_maps(feature1, feature2, W, b)
    res = bass_utils.run_bass_kernel_spmd(nc, in_maps, list(range(N_CORES)))

    # ---- host combine (fp64) ---------------------------------------------
    j = np.arange(L)
    loss_terms = []
    for i in range(N_CORES):
        r = res.results[i]
        # stot[p, m*NCG + bk]; token j = m*128 + p; sum over the NCG blocks
        S = r["s_out"].astype(np.float64).reshape(128, 8, NCG).sum(axis=2)
        S = S.T.reshape(L)                                   # token j at [j%128, j//128]
        t = SHIFT + np.log(S) - np.log(float(N))             # negative_j
        t -= r["pos_main"].astype(np.float64)
        if r_self > 0:
            cnt = np.minimum(L - 1, j + r_self) - np.maximum(0, j - r_self) + 1.0
            t -= r["pos_self"].astype(np.float64) / cnt
        if r_tgt > 0:
            cnt = np.minimum(L - 1, j + r_tgt) - np.maximum(0, j - r_tgt) + 1.0
            t -= r["pos_tgt"].astype(np.float64) / cnt
        loss_terms.append(t)
    loss = np.mean(np.concatenate(loss_terms))
    return np.float32(loss)


def _make_in_maps(feature1, feature2, W, b):
    fp8 = ml_dtypes.float8_e4m3
    f1 = np.asarray(feature1, dtype=np.float32)
    f2 = np.asarray(feature2, dtype=np.float32)
    Wr = np.ascontiguousarray(np.asarray(W, dtype=np.float32).astype(fp8))
    bv = np.ascontiguousarray(np.asarray(b, dtype=np.float32))
    f2t_full = f2.reshape(N, DIN).T.astype(fp8)              # [DIN, N]
    in_maps = []
    for i in range(N_CORES):
        f1t_i = np.ascontiguousarray(f1[i].T.astype(fp8))    # [DIN, L]
        f2t_rot = np.ascontiguousarray(np.roll(f2t_full, -i * L, axis=1))
        in_maps.append({"f1t": f1t_i, "f2t": f2t_rot, "w_in": Wr, "b_in": bv})
    return in_maps


# revision 8
# speedup vs baseline: 2.6450x; 1.4362x over previous
"""Contrastive-learning NCE loss on 8 trn2 NeuronCores (Bass/Tile).

Problem (hardcoded shapes): B=8, L=1024, D_in=512, D_feat=256, N=B*L=8192.
  emb_k = relu(feature_k @ W + b)                     [B, L, Df]
  positive = <e1,e2> + banded_diag_mean terms         [N]
  negative = logsumexp(e1 @ e2.T, axis=-1) - log(N)   [N]
  loss = mean(-positive + negative)

Sharding: token dim N split across 8 cores = one batch row each (L == N/8).
Each core computes its [1024, 8192] slab of the similarity matrix against the
full emb_2 (recomputed locally from full feature2). The host rotates feature2
per core so the core's own batch always sits at columns 0:1023 -> the SPMD
program is core-index free.

v3 design (baseline bf16: ~160-187us; v2 fp8: ~126us):
  * All matmuls fp8e4m3 + DoubleRow (K=256 per MM, 2x bf16 rate; validated
    ~220ns/MM at 512 cols, rel err ~1.5e-4).
  * Constant logsumexp shift (-64): exact for any shift, max sim ~120 keeps
    exp(sim-64) in fp32. exp runs in-place on PSUM with the fused row
    accumulator; host adds 64 + log(S). ACT (ScalarE) is the roofline:
    32 groups x (~1.97us + ~0.29us accum-read) ~= 72us/core.
  * The whole kernel is one software pipeline paced by ACT. The PSUM "mm"
    ring (2 x [128,2048] = all 8 banks) is shared by sim groups and
    projection tiles; projection d-tiles are interleaved INSIDE the sim
    m-loop so every ring slot's consumer (ACT exp or one DVE tensor_scalar)
    has ~2 ACT-groups of slack -- no block-boundary stalls, PE never idles
    >2us (HAM stays warm).
  * bf16 copies of the own-batch embeddings are made from the fp8 SBUF
    tiles (not from PSUM), so each projection PSUM tile has exactly one
    consumer. The copies live in padded [128, NDT, L+2*PAD] tiles (borders
    zeroed once) so the banded box-filter skips its staging step.
  * Banded-term DVE work (copies, box-filters, products) is queued as
    closures and drained one-per-injection-point between projection
    consumers, keeping the DVE from ever blocking the pipeline. The tiny
    PE reduce-MMs for pos_main/self/tgt slot into block 3 (where no
    projection remains), with row evacuation + DMA interleaved after.
"""

import numpy as np
import ml_dtypes
from collections import deque
from contextlib import ExitStack

import concourse.bass as bass
import concourse.tile as tile
from concourse import bacc, mybir
from concourse import bass_utils

dt = mybir.dt
AF = mybir.ActivationFunctionType
ALU = mybir.AluOpType
PM = mybir.MatmulPerfMode

N_CORES = 8
B, L, DIN, DF = 8, 1024, 512, 256
N = B * L
KO = DIN // 128     # 4 k-tiles of the projection contraction
NDT = DF // 128     # 2 d-tiles of the embedding dim
PAD = 4             # box-filter padding (max supported positive_range)
LP = L + 2 * PAD
CW = 2048           # sim-phase column group (one PSUM tile / one ACT)
NCG = N // CW       # 4 sim blocks
SHIFT = 64.0        # constant logsumexp shift

_module_cache = {}


def _box_terms(w: int):
    """Decompose window width w (odd, <= 2*PAD+1) into power-of-2 segments."""
    terms, off = [], 0
    for p in (8, 4, 2, 1):
        if w >= p:
            terms.append((p, off))
            off += p
            w -= p
    assert w == 0
    return terms


def _build(r_self: int, r_tgt: int):
    nc = bacc.Bacc("TRN2", target_bir_lowering=False, debug=False, num_devices=N_CORES)

    f1t = nc.dram_tensor("f1t", [DIN, L], dt.float8e4, kind="ExternalInput").ap()
    f2t = nc.dram_tensor("f2t", [DIN, N], dt.float8e4, kind="ExternalInput").ap()
    w_in = nc.dram_tensor("w_in", [DIN, DF], dt.float8e4, kind="ExternalInput").ap()
    b_in = nc.dram_tensor("b_in", [DF], dt.float32, kind="ExternalInput").ap()

    pos_main = nc.dram_tensor("pos_main", [L], dt.float32, kind="ExternalOutput").ap()
    pos_self = nc.dram_tensor("pos_self", [L], dt.float32, kind="ExternalOutput").ap()
    pos_tgt = nc.dram_tensor("pos_tgt", [L], dt.float32, kind="ExternalOutput").ap()
    s_out = nc.dram_tensor("s_out", [128, 8 * NCG], dt.float32, kind="ExternalOutput").ap()

    with tile.TileContext(nc) as tc, ExitStack() as ctx:
        const = ctx.enter_context(tc.tile_pool(name="const", bufs=1))
        stage = ctx.enter_context(tc.tile_pool(name="stage", bufs=3))
        emb = ctx.enter_context(tc.tile_pool(name="emb", bufs=1))
        band = ctx.enter_context(tc.tile_pool(name="band", bufs=1))
        prodp = ctx.enter_context(tc.tile_pool(name="prodp", bufs=8))
        rows = ctx.enter_context(tc.tile_pool(name="rows", bufs=1))
        mmp = ctx.enter_context(tc.tile_pool(name="mmp", bufs=2, space="PSUM"))

        # ---- constants -------------------------------------------------
        wt = const.tile([128, KO, DF], dt.float8e4)    # W[ko*128+p, d] at [p, ko, d]
        nc.sync.dma_start(out=wt[:], in_=w_in[:].rearrange("(ko p) d -> p ko d", p=128))
        b_col = const.tile([128, NDT], dt.float32)     # bias per (d%128, dtile)
        nc.sync.dma_start(out=b_col[:], in_=b_in[:].rearrange("(d p) -> p d", p=128))
        ones_f = const.tile([128, 1], dt.float32)
        nc.vector.memset(ones_f[:], 1.0)
        ones = const.tile([128, 1], dt.bfloat16)
        nc.vector.tensor_copy(ones[:], ones_f[:])
        neg_shift = const.tile([128, 1], dt.float32)
        nc.vector.memset(neg_shift[:], -SHIFT)
        warm = const.tile([128, 1], dt.float32)
        # dummy exp so the ACT exp-table load happens during the DMA/proj phase
        nc.scalar.activation(warm[:], ones_f[:], AF.Exp, bias=neg_shift[:], scale=1.0)

        # PE warmup burst: ~14 junk matmuls during the DMA head keep the HAM
        # activity monitor busy so the real prologue MMs run at 2.4GHz
        wst = const.tile([128, 128], dt.bfloat16)
        wmv = const.tile([128, 512], dt.bfloat16)
        nc.vector.memset(wst[:], 0.25)
        nc.vector.memset(wmv[:], 0.25)

        # ---- embeddings ------------------------------------------------
        # fp8 k-tile-major layout for DoubleRow: element (p, d, t) = emb[d*128+p, t]
        e1q = emb.tile([128, NDT, L], dt.float8e4, name="e1q", tag="e1q")
        e2q = emb.tile([128, NDT, N], dt.float8e4, name="e2q", tag="e2q")
        # bf16 copies (own batch, PAD-padded for the box filter; borders zeroed)
        e1b = emb.tile([128, NDT, LP], dt.bfloat16, name="e1b", tag="e1b")
        e2b = emb.tile([128, NDT, LP], dt.bfloat16, name="e2b", tag="e2b")
        nc.vector.memzero(e1b[:])
        nc.vector.memzero(e2b[:])

        stot = const.tile([128, 8 * NCG], dt.float32)

        # ---- projection pieces -----------------------------------------
        def pj_mm(src_ap, col0, d, tag):
            """PE half of a projection d-tile: DMA stage (once per chunk) +
            2 accumulating DoubleRow MMs per 512-col half. Returns PSUM tile."""
            nco = 1024
            if d == 0:
                fst = stage.tile([128, KO, nco], dt.float8e4, tag="fstage",
                                 name=f"fst_{tag}")
                nc.sync.dma_start(
                    out=fst[:],
                    in_=src_ap[:, col0:col0 + nco].rearrange("(ko p) n -> p ko n", p=128))
                pj_mm.fst = fst
            fst = pj_mm.fst
            ps = mmp.tile([128, nco], dt.float32, tag="mm", name=f"pj_{tag}_{d}")
            for kk in range(KO // 2):
                for h in range(nco // 512):
                    nc.tensor.matmul(
                        ps[:, h * 512:(h + 1) * 512],
                        wt[:, 2 * kk:2 * kk + 2, d * 128:(d + 1) * 128],
                        fst[:, 2 * kk:2 * kk + 2, h * 512:(h + 1) * 512],
                        start=(kk == 0), stop=(kk == KO // 2 - 1),
                        perf_mode=PM.DoubleRow)
            return ps

        def pj_cons(ps, q_dst, d, q_col0):
            """DVE half: relu(ps + b) -> fp8 destination (single PSUM consumer)."""
            nc.vector.tensor_scalar(
                q_dst[:, d, q_col0:q_col0 + 1024], ps[:],
                b_col[:, d:d + 1], 0.0, ALU.add, ALU.max)

        def project(src_ap, col0, q_dst, q_col0):
            for d in range(NDT):
                ps = pj_mm(src_ap, col0, d, f"c{col0}")
                pj_cons(ps, q_dst, d, q_col0)

        # ---- sim group --------------------------------------------------
        def sim_group(bk, m):
            ps = mmp.tile([128, CW], dt.float32, tag="mm", name=f"sim_{bk}_{m}")
            for q in range(CW // 512):
                nc.tensor.matmul(
                    ps[:, q * 512:(q + 1) * 512],
                    e1q[:, :, m * 128:(m + 1) * 128],
                    e2q[:, :, bk * CW + q * 512: bk * CW + (q + 1) * 512],
                    start=True, stop=True, perf_mode=PM.DoubleRow)
            nc.scalar.activation(ps[:], ps[:], AF.Exp,
                                 bias=neg_shift[:], scale=1.0,
                                 accum_out=stot[:, m * NCG + bk: m * NCG + bk + 1])

        # ---- banded-term closures (drained between pipeline slots) ------
        boxes = {}
        prods = {}

        def mk_copy(dst, src_q, d):
            def f():
                nc.vector.tensor_copy(dst[:, d, PAD:PAD + L], src_q[:, d, 0:L])
            return f

        def mk_boxsum(key, src, d, r):
            """src: padded [128, NDT, LP] tile; result view stored in boxes."""
            def f():
                wdt = 2 * r + 1
                s = {1: src[:, d, :]}
                for p in (2, 4, 8):
                    if wdt >= p:
                        sp = band.tile([128, LP], dt.bfloat16, name=f"s{p}_{key}",
                                       tag=f"s{p}")
                        h = p // 2
                        nv = LP - p + 1
                        nc.vector.tensor_tensor(sp[:, :nv], s[h][:, :nv],
                                                s[h][:, h:h + nv], ALU.add)
                        s[p] = sp
                terms = _box_terms(wdt)
                t0 = PAD - r
                if len(terms) == 1:
                    p0, o0 = terms[0]
                    boxes[key] = s[p0][:, t0 + o0: t0 + o0 + L]
                    return
                acc = band.tile([128, L], dt.bfloat16, name=f"box_{key}",
                                tag="box", bufs=6)
                p0, o0 = terms[0]
                p1, o1 = terms[1]
                nc.vector.tensor_tensor(acc[:], s[p0][:, t0 + o0: t0 + o0 + L],
                                        s[p1][:, t0 + o1: t0 + o1 + L], ALU.add)
                for p, o in terms[2:]:
                    nc.vector.tensor_tensor(acc[:], acc[:],
                                            s[p][:, t0 + o: t0 + o + L], ALU.add)
                boxes[key] = acc[:]
            return f

        def mk_prods(key, pairs_fn):
            def f():
                out = []
                for gi, (a_view, b_view) in enumerate(pairs_fn()):
                    prod = prodp.tile([128, L], dt.bfloat16,
                                      tag=f"prod_{key}_{gi}", bufs=1)
                    nc.vector.tensor_tensor(prod[:], a_view, b_view, ALU.mult)
                    out.append(prod)
                prods[key] = out
            return f

        e1v = [e1b[:, d, PAD:PAD + L] for d in range(NDT)]
        e2v = [e2b[:, d, PAD:PAD + L] for d in range(NDT)]

        dq = deque()
        dq.append(mk_copy(e1b, e1q, 0))
        dq.append(mk_copy(e1b, e1q, 1))
        dq.append(mk_copy(e2b, e2q, 0))
        dq.append(mk_copy(e2b, e2q, 1))
        dq.append(mk_prods("main", lambda: list(zip(e1v, e2v))))
        if r_self:
            for d in range(NDT):
                dq.append(mk_boxsum(("bx1", d), e1b, d, r_self))
            for d in range(NDT):
                dq.append(mk_boxsum(("bx2", d), e2b, d, r_self))
            dq.append(mk_prods("self", lambda: [(e1v[d], boxes[("bx1", d)]) for d in range(NDT)]
                               + [(e2v[d], boxes[("bx2", d)]) for d in range(NDT)]))
        if r_tgt:
            if r_tgt != r_self:
                for d in range(NDT):
                    dq.append(mk_boxsum(("bxt", d), e2b, d, r_tgt))
                tkey = "bxt"
            else:
                tkey = "bx2"
            dq.append(mk_prods("tgt", lambda: [(e1v[d], boxes[(tkey, d)]) for d in range(NDT)]))

        def drain(k=1):
            for _ in range(k):
                if dq:
                    dq.popleft()()

        # ---- pos reduce-MMs + row evacuation (block 3) -------------------
        rps = {}

        def mk_reduce_mm(key):
            def f():
                rp = mmp.tile([1, L], dt.float32, tag="mm", name=f"rp_{key}")
                pr = prods[key]
                for h in range(L // 512):
                    for gi, prod in enumerate(pr):
                        nc.tensor.matmul(rp[:, h * 512:(h + 1) * 512], ones[:],
                                         prod[:, h * 512:(h + 1) * 512],
                                         start=(gi == 0), stop=(gi == len(pr) - 1))
                rps[key] = rp
            return f

        def mk_row(key, out_dram):
            def f():
                row = rows.tile([1, L], dt.float32, tag=f"row_{key}")
                nc.vector.tensor_copy(row[:], rps[key][:])
                nc.sync.dma_start(out=out_dram[:].rearrange("(one n) -> one n", one=1),
                                  in_=row[:])
            return f

        def zero_out(out_dram, tag):
            zr = rows.tile([1, L], dt.float32, tag=f"zr_{tag}")
            nc.vector.memset(zr[:], 0.0)
            nc.sync.dma_start(out=out_dram[:].rearrange("(one n) -> one n", one=1),
                              in_=zr[:])

        red_list = [("main", pos_main)]
        if r_self:
            red_list.append(("self", pos_self))
        else:
            zero_out(pos_self, "self")
        if r_tgt:
            red_list.append(("tgt", pos_tgt))
        else:
            zero_out(pos_tgt, "tgt")

        # ---- schedule ---------------------------------------------------
        # PE warmup (no readers; slots recycled by the prologue)
        for w in range(2):
            wps = mmp.tile([128, 512], dt.float32, tag="mm", name=f"wps_{w}")
            for _ in range(7):
                nc.tensor.matmul(wps[:], wst[:], wmv[:], start=True, stop=True)

        # prologue: project f1 + e2 chunks 0,1 (lean: only the fp8 consumers)
        project(f1t, 0, e1q, 0)
        project(f2t, 0, e2q, 0)
        project(f2t, L, e2q, L)

        # blocks: sim m-loop with projection d-tiles (blocks 0-2) or the pos
        # reduce-MMs and row evacuation (block 3) slotted inside
        for bk in range(NCG):
            mm_slots = {}   # after-m -> PE closure
            dv_slots = {}   # after-m -> DVE closure
            if bk < 3:
                c0, c1 = 2 * bk + 2, 2 * bk + 3
                for idx, (cc, d) in enumerate([(c0, 0), (c0, 1), (c1, 0), (c1, 1)]):
                    m_at = (2, 4, 6, 7)[idx]

                    def mk(cc=cc, d=d):
                        def f():
                            ps = pj_mm(f2t, cc * L, d, f"c{cc}")
                            pj_cons(ps, e2q, d, cc * L)
                            drain(1)
                        return f
                    mm_slots[m_at] = mk()
            else:
                for idx, (key, out_dram) in enumerate(red_list):
                    mm_slots[2 * idx + 1] = mk_reduce_mm(key)
                    dv_slots[2 * idx + 3] = mk_row(key, out_dram)
            for m in range(8):
                sim_group(bk, m)
                if m in mm_slots:
                    mm_slots[m]()
                if m in dv_slots:
                    dv_slots[m]()
        drain(len(dq))   # leftover banded work (normally empty by block 2)

        nc.sync.dma_start(out=s_out[:], in_=stot[:])

    nc.compile()
    return nc


def kernel(feature1, feature2, W, b, positive_range_self, positive_range_tgt):
    r_self = int(np.asarray(positive_range_self))
    r_tgt = int(np.asarray(positive_range_tgt))
    assert 0 <= r_self <= PAD and 0 <= r_tgt <= PAD

    key = (r_self, r_tgt)
    if key not in _module_cache:
        _module_cache[key] = _build(r_self, r_tgt)
    nc = _module_cache[key]

    in_maps = _make_in_maps(feature1, feature2, W, b)
    res = bass_utils.run_bass_kernel_spmd(nc, in_maps, list(range(N_CORES)))

    # ---- host combine (fp64) ---------------------------------------------
    j = np.arange(L)
    loss_terms = []
    for i in range(N_CORES):
        r = res.results[i]
        # stot[p, m*NCG + bk]; token j = m*128 + p; sum over the NCG blocks
        S = r["s_out"].astype(np.float64).reshape(128, 8, NCG).sum(axis=2)
        S = S.T.reshape(L)                                   # token j at [j%128, j//128]
        t = SHIFT + np.log(S) - np.log(float(N))             # negative_j
        t -= r["pos_main"].astype(np.float64)
        if r_self > 0:
            cnt = np.minimum(L - 1, j + r_self) - np.maximum(0, j - r_self) + 1.0
            t -= r["pos_self"].astype(np.float64) / cnt
        if r_tgt > 0:
            cnt = np.minimum(L - 1, j + r_tgt) - np.maximum(0, j - r_tgt) + 1.0
            t -= r["pos_tgt"].astype(np.float64) / cnt
        loss_terms.append(t)
    loss = np.mean(np.concatenate(loss_terms))
    return np.float32(loss)


def _make_in_maps(feature1, feature2, W, b):
    fp8 = ml_dtypes.float8_e4m3
    f1 = np.asarray(feature1, dtype=np.float32)
    f2 = np.asarray(feature2, dtype=np.float32)
    Wr = np.ascontiguousarray(np.asarray(W, dtype=np.float32).astype(fp8))
    bv = np.ascontiguousarray(np.asarray(b, dtype=np.float32))
    f2t_full = f2.reshape(N, DIN).T.astype(fp8)              # [DIN, N]
    in_maps = []
    for i in range(N_CORES):
        f1t_i = np.ascontiguousarray(f1[i].T.astype(fp8))    # [DIN, L]
        f2t_rot = np.ascontiguousarray(np.roll(f2t_full, -i * L, axis=1))
        in_maps.append({"f1t": f1t_i, "f2t": f2t_rot, "w_in": Wr, "b_in": bv})
    return in_maps
